# revision 18
# baseline (speedup 1.0000x reference)
"""Trainium2 Bass kernel for JacobianMLP.

Computes, for x:[B,16], per-head weights W1:[16,512,16], b1:[16,512],
W2:[16,512], b2:[16]:
    h   = einsum('bi,ohi->boh', x, W1) + b1
    h   = leaky_relu(h, 0.2)
    out = einsum('boh,oh->bo', h, W2) + b2

Strategy (8 NeuronCores, data-parallel over batch):
  leaky(h) = 0.2*h + 0.8*relu(h), so
  out = [0.2*W2^T(W1 x + b1) + b2]  (tiny 17x16 folded matmul on x, host)
      + (0.8*W2)^T relu(W1 x + b1)  (main path, device)

The kernel is DRAIN-bound: every h element (8192 x 4096 fp32 per core)
must cross PSUM->SBUF through DVE/ACT at 1 elem/lane/cycle (fp32 PSUM
has no fast DVE mode on trn2). Design:
  L1: 4-way row-tiled CONCURRENT matmuls (tile_position=(32i,0), K=17)
      into a 7-bank PSUM ring, one [128,512] bank per chunk.
  drain: large-FD relu ops over ring pieces of 4 banks (FD=2048) and
      3 banks (FD=1536), alternating ACT (activation Relu) and DVE
      (tensor_scalar_max) in a period-4 pattern that balances engine
      time (ACT 1.2GHz vs DVE 0.96GHz).
  L2: 4-way col-tiled matmuls (M=4 blocks, tile_position=(0,32i)),
      heads grouped so that head h accumulates its FULL 512-wide sum
      in psum partition 32*(h%4) + h//4 of a single acc bank; no
      slice-summing needed.
  out: acc -> SBUF bf16 copy (alternating engines) -> DMA [128,BC].
  DMA triggers stay off the Scalar/Vector queues (sync + gpsimd).
"""

import sys

for _p in ("/opt/trn_rl_repo",):
    if _p not in sys.path:
        sys.path.insert(0, _p)

import numpy as np

B, I, O, H = 32768, 16, 16, 512
NCORES = 8
BC = B // NCORES          # batch per core = 4096
TB = 512                  # batch tile (matmul moving dim)
NT = BC // TB             # batch tiles per core = 8
NROUND = 16               # hid rounds per batch tile (4 chunks each)
NH = O * H                # flat hidden = 8192
NCHUNK = NH // 128        # 64 hid chunks of 128
NBANK = NT * NROUND * 4   # 512 psum bank-writes per core
RING = 7                  # psum ring banks (bank 8 is the L2 acc)
HR = 28                   # hrel SBUF ring slots (4 ring cycles)
LA = 4                    # lookahead (pieces) for L1 emission

_cache = {}


_PSIZE = {0: 2, 2: 2, 4: 2, 6: 1}  # ring pos -> piece size (banks)


def _pieces():
    """Drain pieces: (start bank g0, size). Ring cycle = 3 + 3 + 1
    banks; the two 3-bank slots live in one bufs=2 pool and engines
    alternate per piece, so each slot's L1 refill (plus its semaphore
    chain) hides under the other slots' drains."""
    out = []
    g0 = 0
    while g0 < NBANK:
        size = min(_PSIZE[g0 % RING], NBANK - g0)
        out.append((g0, size))
        g0 += size
    return out


def _build():
    key = "nc"
    if key in _cache:
        return _cache[key]

    import concourse.bacc as bacc
    import concourse.tile as tile
    from concourse import mybir

    f32 = mybir.dt.float32
    bf16 = mybir.dt.bfloat16
    Relu = mybir.ActivationFunctionType.Relu

    nc = bacc.Bacc(
        "TRN2",
        target_bir_lowering=False,
        debug=False,
        num_devices=NCORES,
    )

    xr_d = nc.dram_tensor("xr", [128, BC], bf16, kind="ExternalInput")
    w1s_d = nc.dram_tensor("w1s", [128, NCHUNK * 128], bf16, kind="ExternalInput")
    w2s_d = nc.dram_tensor("w2s", [128, NCHUNK * 32], bf16, kind="ExternalInput")
    y_d = nc.dram_tensor("y", [128, BC], bf16, kind="ExternalOutput")

    with tile.TileContext(nc) as tc:
        with (
            tc.tile_pool(name="consts", bufs=1) as consts,
            tc.tile_pool(name="xp", bufs=4) as xp,
            tc.tile_pool(name="stkp", bufs=2) as stkp,
            tc.tile_pool(name="h2p", bufs=12) as h2p,
            tc.tile_pool(name="h1p", bufs=6) as h1p,
            tc.tile_pool(name="warm", bufs=1) as warm,
            tc.tile_pool(name="accp", bufs=1, space="PSUM") as accp,
            tc.tile_pool(name="pqa", bufs=1, space="PSUM") as pqa,
            tc.tile_pool(name="pqb", bufs=1, space="PSUM") as pqb,
            tc.tile_pool(name="pqc", bufs=1, space="PSUM") as pqc,
            tc.tile_pool(name="pqd", bufs=1, space="PSUM") as pqd,
        ):
            w1sb = consts.tile([128, NCHUNK * 128], bf16, name="w1sb")
            w2sb = consts.tile([128, NCHUNK * 32], bf16, name="w2sb")

            pieces = _pieces()
            pidx = {}
            for p, (g0, size) in enumerate(pieces):
                for k in range(size):
                    pidx[g0 + k] = (p, k)

            xts = {}
            accs = {}
            ptiles = {}
            htiles = {}

            _SLOT = {0: None, 2: None, 4: None, 6: None}

            def get_ptile(p):
                if p not in ptiles:
                    pos = pieces[p][0] % RING
                    if pos == 0:
                        ptiles[p] = pqa.tile([128, 2 * TB], f32, name="pa", tag="pa")
                    elif pos == 2:
                        ptiles[p] = pqb.tile([128, 2 * TB], f32, name="pb", tag="pb")
                    elif pos == 4:
                        ptiles[p] = pqc.tile([128, 2 * TB], f32, name="pc", tag="pc")
                    else:
                        ptiles[p] = pqd.tile([128, TB], f32, name="pd", tag="pd")
                return ptiles[p]

            def make_xt(bt):
                xt = xp.tile([128, TB], bf16, name="xt", tag="xt")
                nc.sync.dma_start(xt[:], xr_d[:, bt * TB : (bt + 1) * TB])
                xts[bt] = xt

            # preload the ACT table set (~2.7us) during the DMA prologue
            wt = warm.tile([128, 8], f32, name="wt")
            nc.vector.memset(wt[:], 0.0)
            nc.scalar.activation(wt[:, 4:8], wt[:, 0:4], Relu)

            # first-needed weights on the (otherwise idle) gpsimd queue so
            # the sync queue can deliver x tiles unimpeded; w1s streams in
            # round order so bt0's sweep never starves
            nc.gpsimd.dma_start(w1sb[:, 0:512], w1s_d[:, 0:512])
            make_xt(0)
            make_xt(1)
            nc.gpsimd.dma_start(w2sb[:, 0:256], w2s_d[:, 0:256])
            nc.gpsimd.dma_start(w1sb[:, 512:1536], w1s_d[:, 512:1536])
            nc.gpsimd.dma_start(w2sb[:, 256:2048], w2s_d[:, 256:2048])
            nc.gpsimd.dma_start(w1sb[:, 1536:4864], w1s_d[:, 1536:4864])
            nc.sync.dma_start(w1sb[:, 4864:8192], w1s_d[:, 4864:8192])

            def emit_l1(g):
                bt, rr, i = g // 64, (g // 4) % 16, g % 4
                if i == 0 and rr == 0:
                    if bt + 2 < NT:
                        make_xt(bt + 2)
                    accs[bt] = accp.tile([128, TB], f32, name="acc", tag="acc")
                xt = xts[bt]
                p, off = pidx[g]
                out = get_ptile(p)[:, TB * off : TB * off + TB]
                c = 4 * rr + i
                nc.tensor.matmul(
                    out,
                    w1sb[:, 128 * c : 128 * c + 128],
                    xt[:, :],
                    start=True,
                    stop=True,
                )

            def emit_l2(g):
                bt, rr, i = g // 64, (g // 4) % 16, g % 4
                c = 32 * (4 * rr + i)
                p, off = pidx[g]
                nc.tensor.matmul(
                    accs[bt][32 * i : 32 * i + 32, :],
                    w2sb[:, c : c + 32],
                    htiles[p][:, TB * off : TB * off + TB],
                    start=(rr == 0),
                    stop=(rr == 15),
                    tile_position=(0, 32 * i),
                    skip_group_check=True,
                )

            def emit_l2_round(g3):
                # full round quad: 4 col-tiled MMs issue back-to-back so
                # they stream concurrently on the 4 col-group xbuses
                for g in range(g3 - 3, g3 + 1):
                    emit_l2(g)
                if g3 % 64 == 63:
                    emit_tail(g3 // 64)

            def emit_tail(bt):
                acc = accs.pop(bt)
                stk = stkp.tile([128, TB], bf16, name="stk", tag="stk")
                nc.scalar.copy(stk[:], acc[:])
                nc.gpsimd.dma_start(y_d[:, bt * TB : (bt + 1) * TB], stk[:])

            emitted = 0
            pending_l2 = []
            for p, (g0, size) in enumerate(pieces):
                hp, hs = pieces[min(p + LA, len(pieces) - 1)]
                while emitted < hp + hs:
                    emit_l1(emitted)
                    emitted += 1
                src = ptiles[p][:, 0 : TB * size]
                pos = g0 % RING
                if pos != 6:
                    ht = h2p.tile([128, 2 * TB], bf16, name="h2", tag="h2")
                else:
                    ht = h1p.tile([128, TB], bf16, name="h1", tag="h1")
                htiles[p] = ht
                dst = ht[:, 0 : TB * size]
                m = g0 // RING
                use_act = (m % 2 == 0) != (pos == 2)
                if use_act:
                    nc.scalar.activation(dst, src, Relu)
                else:
                    nc.vector.tensor_scalar_max(dst, src, 0.0)
                # emit round-quads >=1 piece late (drain sems already
                # satisfied -> no FIFO block) and in pairs so consecutive
                # quads pipeline on the array (~227ns instead of ~400ns)
                if len(pending_l2) >= 4:
                    for g3 in pending_l2:
                        emit_l2_round(g3)
                    pending_l2 = []
                pending_l2 += [g for g in range(g0, g0 + size) if g % 4 == 3]
                ptiles.pop(p)
            for g3 in pending_l2:
                emit_l2_round(g3)

    nc.compile()
    _cache[key] = nc
    return nc


def _prep_inputs(x, W1, b1, W2, b2):
    """Build per-core in_maps (host-side shard + weight folding)."""
    import ml_dtypes

    x = np.asarray(x, dtype=np.float32)
    W1 = np.asarray(W1, dtype=np.float32)
    b1 = np.asarray(b1, dtype=np.float32)
    W2 = np.asarray(W2, dtype=np.float32)
    b2 = np.asarray(b2, dtype=np.float32)

    W1f = W1.reshape(NH, I)              # [8192, 16]
    b1f = b1.reshape(NH)                 # [8192]

    # w1s: one full [128,128] stationary per chunk c=4rr+i (head 4a+i,
    # quarter q): rows 32i..32i+16 hold W1^T, row 32i+16 holds b1
    w1s = np.zeros((128, NCHUNK * 128), dtype=np.float32)
    # w2s: per (rr, i): [128, 32] block, col a = 0.8 * W2[head, quarter]
    w2s = np.zeros((128, NCHUNK * 32), dtype=np.float32)
    for rr in range(NROUND):
        a, q = rr // 4, rr % 4
        for i in range(4):
            head = 4 * a + i
            hb = head * 512 + q * 128
            c = 4 * rr + i
            csl = slice(128 * c, 128 * c + 128)
            w1s[32 * i : 32 * i + 16, csl] = W1f[hb : hb + 128].T
            w1s[32 * i + 16, csl] = b1f[hb : hb + 128]
            w2s[:, 32 * c + a] = 0.8 * W2[head, 128 * q : 128 * q + 128]
    w1s = w1s.astype(ml_dtypes.bfloat16)
    w2s = w2s.astype(ml_dtypes.bfloat16)

    # exact linear path done host-side: 0.2 * W2^T (W1 x + b1) + b2
    linw = np.zeros((16, 16), dtype=np.float32)   # [i, o]
    linb = np.zeros((16,), dtype=np.float32)
    for o in range(O):
        linw[:, o] = 0.2 * (W2[o] @ W1[o])
        linb[o] = 0.2 * float(W2[o] @ b1[o]) + float(b2[o])

    in_maps = []
    for core in range(NCORES):
        xc = x[core * BC : (core + 1) * BC]          # [4096, 16]
        xa = np.zeros((32, BC), dtype=np.float32)
        xa[0:16] = xc.T
        xa[16] = 1.0
        xr = np.tile(xa, (4, 1)).astype(ml_dtypes.bfloat16)  # [128, 4096]
        in_maps.append(
            {
                "xr": np.ascontiguousarray(xr),
                "w1s": w1s,
                "w2s": w2s,
            }
        )
    return in_maps, linw, linb


# head h's full sum lives in psum/y partition 32*(h%4) + h//4
_YROWS = [32 * (h % 4) + h // 4 for h in range(O)]


def _unshard_core(yp, xc, linw, linb):
    rel = yp[_YROWS].astype(np.float32)              # [16, BC]
    return rel.T + xc @ linw + linb


last_results = None


def kernel(x, W1, b1, W2, b2):
    global last_results
    from concourse.bass_utils import run_bass_kernel_spmd

    nc = _build()
    in_maps, linw, linb = _prep_inputs(x, W1, b1, W2, b2)
    res = run_bass_kernel_spmd(nc, in_maps, core_ids=list(range(NCORES)))
    last_results = res
    x = np.asarray(x, dtype=np.float32)
    out = np.empty((B, O), dtype=np.float32)
    for core in range(NCORES):
        yp = np.asarray(res.results[core]["y"])      # [128, BC] bf16
        xc = x[core * BC : (core + 1) * BC]
        out[core * BC : (core + 1) * BC] = _unshard_core(yp, xc, linw, linb)
    return out


# revision 19
# speedup vs baseline: 1.1310x; 1.1310x over previous
"""Trainium2 Bass kernel for JacobianMLP.

Computes, for x:[B,16], per-head weights W1:[16,512,16], b1:[16,512],
W2:[16,512], b2:[16]:
    h   = einsum('bi,ohi->boh', x, W1) + b1
    h   = leaky_relu(h, 0.2)
    out = einsum('boh,oh->bo', h, W2) + b2

Strategy (8 NeuronCores, data-parallel over batch):
  leaky(h) = 0.2*h + 0.8*relu(h), so
  out = [0.2*W2^T(W1 x + b1) + b2]  (tiny 17x16 folded matmul on x, host)
      + (0.8*W2)^T relu(W1 x + b1)  (main path, device)

The kernel is DRAIN-bound: every h element (8192 x 4096 fp32 per core)
must cross PSUM->SBUF through DVE/ACT at 1 elem/lane/cycle (fp32 PSUM
has no fast DVE mode on trn2). Design:
  L1: 4-way row-tiled CONCURRENT matmuls (tile_position=(32i,0), K=17)
      into a 7-bank PSUM ring, one [128,512] bank per chunk.
  drain: large-FD relu ops over ring pieces of 4 banks (FD=2048) and
      3 banks (FD=1536), alternating ACT (activation Relu) and DVE
      (tensor_scalar_max) in a period-4 pattern that balances engine
      time (ACT 1.2GHz vs DVE 0.96GHz).
  L2: 4-way col-tiled matmuls (M=4 blocks, tile_position=(0,32i)),
      heads grouped so that head h accumulates its FULL 512-wide sum
      in psum partition 32*(h%4) + h//4 of a single acc bank; no
      slice-summing needed.
  out: acc -> SBUF bf16 copy (alternating engines) -> DMA [128,BC].
  DMA triggers stay off the Scalar/Vector queues (sync + gpsimd).
"""

import sys

for _p in ("/opt/trn_rl_repo",):
    if _p not in sys.path:
        sys.path.insert(0, _p)

import numpy as np

B, I, O, H = 32768, 16, 16, 512
NCORES = 8
BC = B // NCORES          # batch per core = 4096
TB = 512                  # batch tile (matmul moving dim)
NT = BC // TB             # batch tiles per core = 8
NROUND = 16               # hid rounds per batch tile (4 chunks each)
NH = O * H                # flat hidden = 8192
NCHUNK = NH // 128        # 64 hid chunks of 128
NBANK = NT * NROUND * 4   # 512 psum bank-writes per core
RING = 7                  # psum ring banks (bank 8 is the L2 acc)
HR = 28                   # hrel SBUF ring slots (4 ring cycles)
LA = 3                    # lookahead (pieces) for L1 emission

_cache = {}


_PSIZE = {0: 2, 2: 2, 4: 2, 6: 1}  # ring pos -> piece size (banks)


def _pieces():
    """Drain pieces: (start bank g0, size). Ring cycle = 3 + 3 + 1
    banks; the two 3-bank slots live in one bufs=2 pool and engines
    alternate per piece, so each slot's L1 refill (plus its semaphore
    chain) hides under the other slots' drains."""
    out = []
    g0 = 0
    while g0 < NBANK:
        size = min(_PSIZE[g0 % RING], NBANK - g0)
        out.append((g0, size))
        g0 += size
    return out


def _build():
    key = "nc"
    if key in _cache:
        return _cache[key]

    import concourse.bacc as bacc
    import concourse.tile as tile
    from concourse import mybir

    f32 = mybir.dt.float32
    bf16 = mybir.dt.bfloat16
    Relu = mybir.ActivationFunctionType.Relu

    nc = bacc.Bacc(
        "TRN2",
        target_bir_lowering=False,
        debug=False,
        num_devices=NCORES,
    )

    xr_d = nc.dram_tensor("xr", [128, BC], bf16, kind="ExternalInput")
    w1s_d = nc.dram_tensor("w1s", [128, NCHUNK * 128], bf16, kind="ExternalInput")
    w2s_d = nc.dram_tensor("w2s", [128, NCHUNK * 32], bf16, kind="ExternalInput")
    y_d = nc.dram_tensor("y", [128, BC], bf16, kind="ExternalOutput")

    with tile.TileContext(nc) as tc:
        with (
            tc.tile_pool(name="consts", bufs=1) as consts,
            tc.tile_pool(name="xp", bufs=4) as xp,
            tc.tile_pool(name="stkp", bufs=2) as stkp,
            tc.tile_pool(name="h2p", bufs=12) as h2p,
            tc.tile_pool(name="h1p", bufs=6) as h1p,
            tc.tile_pool(name="warm", bufs=1) as warm,
            tc.tile_pool(name="accp", bufs=1, space="PSUM") as accp,
            tc.tile_pool(name="pqa", bufs=1, space="PSUM") as pqa,
            tc.tile_pool(name="pqb", bufs=1, space="PSUM") as pqb,
            tc.tile_pool(name="pqc", bufs=1, space="PSUM") as pqc,
            tc.tile_pool(name="pqd", bufs=1, space="PSUM") as pqd,
        ):
            w1sb = consts.tile([128, NCHUNK * 128], bf16, name="w1sb")
            w2sb = consts.tile([128, NCHUNK * 32], bf16, name="w2sb")

            pieces = _pieces()
            pidx = {}
            for p, (g0, size) in enumerate(pieces):
                for k in range(size):
                    pidx[g0 + k] = (p, k)

            xts = {}
            accs = {}
            ptiles = {}
            htiles = {}

            _SLOT = {0: None, 2: None, 4: None, 6: None}

            def get_ptile(p):
                if p not in ptiles:
                    pos = pieces[p][0] % RING
                    if pos == 0:
                        ptiles[p] = pqa.tile([128, 2 * TB], f32, name="pa", tag="pa")
                    elif pos == 2:
                        ptiles[p] = pqb.tile([128, 2 * TB], f32, name="pb", tag="pb")
                    elif pos == 4:
                        ptiles[p] = pqc.tile([128, 2 * TB], f32, name="pc", tag="pc")
                    else:
                        ptiles[p] = pqd.tile([128, TB], f32, name="pd", tag="pd")
                return ptiles[p]

            def make_xt(bt):
                xt = xp.tile([128, TB], bf16, name="xt", tag="xt")
                nc.sync.dma_start(xt[:], xr_d[:, bt * TB : (bt + 1) * TB])
                xts[bt] = xt

            # preload the ACT table set (~2.7us) during the DMA prologue
            wt = warm.tile([128, 8], f32, name="wt")
            nc.vector.memset(wt[:], 0.0)
            nc.scalar.activation(wt[:, 4:8], wt[:, 0:4], Relu)

            # first-needed weights on the (otherwise idle) gpsimd queue so
            # the sync queue can deliver x tiles unimpeded; w1s streams in
            # round order so bt0's sweep never starves
            nc.gpsimd.dma_start(w1sb[:, 0:512], w1s_d[:, 0:512])
            make_xt(0)
            make_xt(1)
            nc.gpsimd.dma_start(w2sb[:, 0:256], w2s_d[:, 0:256])
            nc.gpsimd.dma_start(w1sb[:, 512:1536], w1s_d[:, 512:1536])
            nc.gpsimd.dma_start(w2sb[:, 256:2048], w2s_d[:, 256:2048])
            nc.gpsimd.dma_start(w1sb[:, 1536:4864], w1s_d[:, 1536:4864])
            nc.sync.dma_start(w1sb[:, 4864:8192], w1s_d[:, 4864:8192])

            def emit_l1(g):
                bt, rr, i = g // 64, (g // 4) % 16, g % 4
                if i == 0 and rr == 0:
                    if bt + 2 < NT:
                        make_xt(bt + 2)
                    accs[bt] = accp.tile([128, TB], f32, name="acc", tag="acc")
                xt = xts[bt]
                p, off = pidx[g]
                out = get_ptile(p)[:, TB * off : TB * off + TB]
                c = 4 * rr + i
                nc.tensor.matmul(
                    out,
                    w1sb[:, 128 * c : 128 * c + 128],
                    xt[:, :],
                    start=True,
                    stop=True,
                )

            def emit_l2(g):
                bt, rr, i = g // 64, (g // 4) % 16, g % 4
                c = 32 * (4 * rr + i)
                p, off = pidx[g]
                nc.tensor.matmul(
                    accs[bt][32 * i : 32 * i + 32, :],
                    w2sb[:, c : c + 32],
                    htiles[p][:, TB * off : TB * off + TB],
                    start=(rr == 0),
                    stop=(rr == 15),
                    tile_position=(0, 32 * i),
                    skip_group_check=True,
                )

            def emit_l2_round(g3):
                # full round quad: 4 col-tiled MMs issue back-to-back so
                # they stream concurrently on the 4 col-group xbuses
                for g in range(g3 - 3, g3 + 1):
                    emit_l2(g)
                if g3 % 64 == 63:
                    emit_tail(g3 // 64)

            def emit_tail(bt):
                acc = accs.pop(bt)
                stk = stkp.tile([128, TB], bf16, name="stk", tag="stk")
                nc.scalar.copy(stk[:], acc[:])
                nc.gpsimd.dma_start(y_d[:, bt * TB : (bt + 1) * TB], stk[:])

            emitted = 0
            pending_l2 = []
            for p, (g0, size) in enumerate(pieces):
                # make sure this piece's L1 is emitted before its drain
                while emitted < g0 + size:
                    emit_l1(emitted)
                    emitted += 1
                src = ptiles[p][:, 0 : TB * size]
                pos = g0 % RING
                if pos != 6:
                    ht = h2p.tile([128, 2 * TB], bf16, name="h2", tag="h2")
                else:
                    ht = h1p.tile([128, TB], bf16, name="h1", tag="h1")
                htiles[p] = ht
                dst = ht[:, 0 : TB * size]
                m = g0 // RING
                use_act = (m % 2 == 0) != (pos == 2)
                if use_act:
                    nc.scalar.activation(dst, src, Relu)
                else:
                    nc.vector.tensor_scalar_max(dst, src, 0.0)
                # emit round-quads >=1 piece late (drain sems already
                # satisfied -> no FIFO block) and in pairs so consecutive
                # quads pipeline on the array (~227ns instead of ~400ns)
                if len(pending_l2) >= 2:
                    for g3 in pending_l2:
                        emit_l2_round(g3)
                    pending_l2 = []
                pending_l2 += [g for g in range(g0, g0 + size) if g % 4 == 3]
                # L1 lookahead last: a WAR-blocked L1 never hides ready work
                hp, hs = pieces[min(p + LA, len(pieces) - 1)]
                while emitted < hp + hs:
                    emit_l1(emitted)
                    emitted += 1
                ptiles.pop(p)
            for g3 in pending_l2:
                emit_l2_round(g3)

    nc.compile()
    _cache[key] = nc
    return nc


def _prep_inputs(x, W1, b1, W2, b2):
    """Build per-core in_maps (host-side shard + weight folding)."""
    import ml_dtypes

    x = np.asarray(x, dtype=np.float32)
    W1 = np.asarray(W1, dtype=np.float32)
    b1 = np.asarray(b1, dtype=np.float32)
    W2 = np.asarray(W2, dtype=np.float32)
    b2 = np.asarray(b2, dtype=np.float32)

    W1f = W1.reshape(NH, I)              # [8192, 16]
    b1f = b1.reshape(NH)                 # [8192]

    # w1s: one full [128,128] stationary per chunk c=4rr+i (head 4a+i,
    # quarter q): rows 32i..32i+16 hold W1^T, row 32i+16 holds b1
    w1s = np.zeros((128, NCHUNK * 128), dtype=np.float32)
    # w2s: per (rr, i): [128, 32] block, col a = 0.8 * W2[head, quarter]
    w2s = np.zeros((128, NCHUNK * 32), dtype=np.float32)
    for rr in range(NROUND):
        a, q = rr // 4, rr % 4
        for i in range(4):
            head = 4 * a + i
            hb = head * 512 + q * 128
            c = 4 * rr + i
            csl = slice(128 * c, 128 * c + 128)
            w1s[32 * i : 32 * i + 16, csl] = W1f[hb : hb + 128].T
            w1s[32 * i + 16, csl] = b1f[hb : hb + 128]
            w2s[:, 32 * c + a] = 0.8 * W2[head, 128 * q : 128 * q + 128]
    w1s = w1s.astype(ml_dtypes.bfloat16)
    w2s = w2s.astype(ml_dtypes.bfloat16)

    # exact linear path done host-side: 0.2 * W2^T (W1 x + b1) + b2
    linw = np.zeros((16, 16), dtype=np.float32)   # [i, o]
    linb = np.zeros((16,), dtype=np.float32)
    for o in range(O):
        linw[:, o] = 0.2 * (W2[o] @ W1[o])
        linb[o] = 0.2 * float(W2[o] @ b1[o]) + float(b2[o])

    in_maps = []
    for core in range(NCORES):
        xc = x[core * BC : (core + 1) * BC]          # [4096, 16]
        xa = np.zeros((32, BC), dtype=np.float32)
        xa[0:16] = xc.T
        xa[16] = 1.0
        xr = np.tile(xa, (4, 1)).astype(ml_dtypes.bfloat16)  # [128, 4096]
        in_maps.append(
            {
                "xr": np.ascontiguousarray(xr),
                "w1s": w1s,
                "w2s": w2s,
            }
        )
    return in_maps, linw, linb


# head h's full sum lives in psum/y partition 32*(h%4) + h//4
_YROWS = [32 * (h % 4) + h // 4 for h in range(O)]


def _unshard_core(yp, xc, linw, linb):
    rel = yp[_YROWS].astype(np.float32)              # [16, BC]
    return rel.T + xc @ linw + linb


last_results = None


def kernel(x, W1, b1, W2, b2):
    global last_results
    from concourse.bass_utils import run_bass_kernel_spmd

    nc = _build()
    in_maps, linw, linb = _prep_inputs(x, W1, b1, W2, b2)
    res = run_bass_kernel_spmd(nc, in_maps, core_ids=list(range(NCORES)))
    last_results = res
    x = np.asarray(x, dtype=np.float32)
    out = np.empty((B, O), dtype=np.float32)
    for core in range(NCORES):
        yp = np.asarray(res.results[core]["y"])      # [128, BC] bf16
        xc = x[core * BC : (core + 1) * BC]
        out[core * BC : (core + 1) * BC] = _unshard_core(yp, xc, linw, linb)
    return out
